# revision 17
# baseline (speedup 1.0000x reference)
"""BCM_Conv2d_fft kernel for Trainium2 (8 NeuronCores, batch-parallel).

The reference computes a block-circulant 3x3 conv via per-block rfft/irfft.
That is mathematically identical to a standard 3x3 convolution with the
expanded block-circulant weight matrix
    W_full[k*8+t, p*8+n] = w[p, k, (n-t) % 8]
so on device we run a plain conv: out = W_full.T @ im2col(x) + b,
implemented as 9 spatially-shifted matmuls accumulating in PSUM.

Sharding: data-parallel over batch B=8 -> one image per core; the small
weight is expanded on host and replicated to all cores. x is pre-padded on
host so the device input DMA is a single contiguous copy per 128-channel
chunk.
"""

import os

import numpy as np

import concourse.bacc as bacc
import concourse.bass as bass
import concourse.mybir as mybir
import concourse.tile as tile
from concourse.bass import ts, _add_dep_helper
from concourse.bass_utils import run_bass_kernel_spmd

N_CORES = 8
C = 256          # channels in = out
H = W = 56
HP = H + 2       # padded spatial
KK = 3           # conv kernel
BS = 8           # circulant block size
L = H * W        # 3136 pixels
ROWS_PER_TILE = 8
NT = ROWS_PER_TILE * W   # 448 pixels per N tile
N_TILES = L // NT        # 7
KCH = (KK * KK * C) // 128   # 18 K-chunks of 128
MCH = C // 128               # 2 M-chunks of 128

F32 = mybir.dt.float32

LAST_RESULT = None  # BassKernelResults of the most recent kernel() call


def _expand_weight(w: np.ndarray) -> np.ndarray:
    """w [32, 288, 8] -> SBUF-layout lhsT blocks [128, 36*128].

    W_full[k*8+t, p*8+n] = w[p, k, (n-t) % 8]; block (ki, m) at columns
    (ki*MCH+m)*128 holds W_full[ki*128+p, m*128+f].
    """
    num_r, num_c, bs = w.shape
    wfull = np.zeros((num_c * bs, num_r * bs), np.float32)
    for t in range(bs):
        for n in range(bs):
            wfull[t::bs, n::bs] = w[:, :, (n - t) % bs].T
    return np.ascontiguousarray(
        wfull.reshape(KCH, 128, MCH, 128)
        .transpose(1, 0, 2, 3)
        .reshape(128, KCH * MCH * 128)
    )


def _kernel_body(tc, x, wt, bias, out):
    nc = tc.nc
    with (
        tc.tile_pool(name="const", bufs=1) as const_pool,
        tc.tile_pool(name="xp", bufs=1) as xp_pool,
        tc.tile_pool(name="ob", bufs=4) as out_pool,
        tc.tile_pool(name="ps", bufs=4, space="PSUM") as psum_pool,
        tc.tile_pool(name="dps", bufs=1, space="PSUM") as dummy_pool,
    ):
        # All 36 stationary blocks side by side: [128, 36*128]
        wt_sb = const_pool.tile([128, KCH * MCH * 128], F32)
        nc.gpsimd.dma_start(out=wt_sb[:], in_=wt[:, :])

        bias_sb = const_pool.tile([128, MCH], F32)
        nc.gpsimd.dma_start(out=bias_sb[:], in_=bias[:, :])

        # Pre-padded input, one tile per 128-channel chunk (contiguous DMA)
        xq = []
        for i in range(MCH):
            t = xp_pool.tile([128, HP * HP], F32, tag=f"xp{i}")
            nc.gpsimd.dma_start(
                out=t[:], in_=x[ts(i, 128), :, :].rearrange("p h w -> p (h w)")
            )
            xq.append(t)

        # Wait-absorber matmuls: walrus allows a single sync wait on a
        # Matmult (fp32 LDWEIGHTS slot). Chain one tiny matmul per input
        # DMA so each PE instruction picks up exactly one new semaphore;
        # all real matmuls then start with their inputs already observed.
        dummies = []
        dps = dummy_pool.tile([128, 64], F32)
        for src_t in (xq[0], xq[1], wt_sb):
            d = nc.tensor.matmul(
                dps[0:1, 0:64],
                lhsT=src_t[:, 0:1],
                rhs=src_t[:, 0:64],
                start=True,
                stop=True,
            )
            if dummies:
                _add_dep_helper(
                    d.ins, dummies[-1].ins, sync=False,
                    reason="absorber chain",
                )
            dummies.append(d)

        # Same trick for the scalar engine: absorb the bias-DMA wait.
        junk = out_pool.tile([128, 1], F32, tag="junk")
        dact = nc.scalar.activation(
            junk[:], bias_sb[:, 0:1], mybir.ActivationFunctionType.Identity,
        )

        def conv_tile(nt, ob):
            """Compute output row-tile nt into ob [128, MCH, NT]."""
            for m in range(MCH):
                ps = psum_pool.tile([128, NT], F32, tag="ps")
                n_mm = 0
                for kh in range(KK):
                    for kw in range(KK):
                        for cb in range(MCH):
                            ki = (kh * KK + kw) * MCH + cb
                            xv = xq[cb][:].rearrange("p (h w) -> p h w", h=HP)
                            rhs = xv[
                                :,
                                nt * ROWS_PER_TILE + kh:
                                nt * ROWS_PER_TILE + kh + ROWS_PER_TILE,
                                kw: kw + W,
                            ]
                            mm = nc.tensor.matmul(
                                ps[:],
                                lhsT=wt_sb[:, ts(ki * MCH + m, 128)],
                                rhs=rhs,
                                start=(n_mm == 0),
                                stop=(n_mm == KCH - 1),
                            )
                            if n_mm == 0:
                                _add_dep_helper(
                                    mm.ins, dummies[-1].ins, sync=False,
                                    reason="after absorbers",
                                )
                            n_mm += 1
                act = nc.scalar.activation(
                    ob[:, m, :], ps[:],
                    mybir.ActivationFunctionType.Identity,
                    bias=bias_sb[:, m: m + 1],
                )
                _add_dep_helper(
                    act.ins, dact.ins, sync=False, reason="after bias absorber",
                )

        # out viewed as [p(128), m(2), pix]: c = m*128 + p
        out_v = out.rearrange("(m p) h w -> p m (h w)", m=MCH)
        # Group row-tiles into 4 output DMAs so (with the 4 input DMAs) at
        # most 8 DMA queue procs are ever used: no queue reuse, one wait per
        # descriptor, and a tail drain the CTRL struct can encode.
        for grp in ((0, 1), (2, 3), (4, 5), (6,)):
            gob = out_pool.tile(
                [128, MCH, len(grp) * NT], F32, tag=f"ob{len(grp)}"
            )
            for gi, nt in enumerate(grp):
                conv_tile(nt, gob[:, :, gi * NT: (gi + 1) * NT])
            nc.scalar.dma_start(
                out=out_v[:, :, grp[0] * NT: (grp[-1] + 1) * NT], in_=gob[:]
            )


def _build_nc():
    nc = bacc.Bacc("TRN2", target_bir_lowering=False, debug=False)
    x = nc.dram_tensor("x", [C, HP, HP], F32, kind="ExternalInput").ap()
    wt = nc.dram_tensor("wt", [128, KCH * MCH * 128], F32, kind="ExternalInput").ap()
    bias = nc.dram_tensor("bias", [128, MCH], F32, kind="ExternalInput").ap()
    out = nc.dram_tensor("out", [C, H, W], F32, kind="ExternalOutput").ap()
    with tile.TileContext(nc) as tc:
        _kernel_body(tc, x, wt, bias, out)
    nc.compile()
    return nc


def kernel(x: np.ndarray, w: np.ndarray, b: np.ndarray) -> np.ndarray:
    global LAST_RESULT
    xp = np.pad(np.asarray(x, np.float32), ((0, 0), (0, 0), (1, 1), (1, 1)))
    xp = np.ascontiguousarray(xp)
    wt = _expand_weight(np.asarray(w, np.float32))
    b = np.ascontiguousarray(
        np.asarray(b, np.float32).reshape(MCH, 128).T
    )

    nc = _build_nc()
    in_maps = [{"x": xp[i], "wt": wt, "bias": b} for i in range(N_CORES)]
    res = run_bass_kernel_spmd(
        nc,
        in_maps,
        core_ids=list(range(N_CORES)),
        trace=bool(int(os.environ.get("KERNEL_PROFILE", "0"))),
    )
    LAST_RESULT = res
    return np.stack([res.results[i]["out"] for i in range(N_CORES)], axis=0)


# revision 20
# speedup vs baseline: 2.5781x; 2.5781x over previous
"""BCM_Conv2d_fft kernel for Trainium2 (8 NeuronCores, batch-parallel).

The reference computes a block-circulant 3x3 conv via per-block rfft/irfft.
That is mathematically identical to a standard 3x3 convolution with the
expanded block-circulant weight matrix
    W_full[k*8+t, p*8+n] = w[p, k, (n-t) % 8]
so on device we run a plain conv: out = W_full.T @ im2col(x) + b,
implemented as 9 spatially-shifted matmuls accumulating in PSUM.

Sharding: data-parallel over batch B=8 -> one image per core; the small
weight is expanded on host and replicated to all cores. x is pre-padded on
host so the device input DMA is a single contiguous copy per 128-channel
chunk.
"""

import os

import numpy as np

import concourse.bacc as bacc
import concourse.bass as bass
import concourse.mybir as mybir
import concourse.tile as tile
from concourse.bass import ts, _add_dep_helper
from concourse.bass_utils import run_bass_kernel_spmd

N_CORES = 8
C = 256          # channels in = out
H = W = 56
HP = H + 2       # padded spatial
KK = 3           # conv kernel
BS = 8           # circulant block size
L = H * W        # 3136 pixels
ROWS_PER_TILE = 8
NT = ROWS_PER_TILE * W   # 448 pixels per N tile
N_TILES = L // NT        # 7
KCH = (KK * KK * C) // 128   # 18 K-chunks of 128
MCH = C // 128               # 2 M-chunks of 128

F32 = mybir.dt.float32
F32R = mybir.dt.float32r

LAST_RESULT = None  # BassKernelResults of the most recent kernel() call


def _expand_weight(w: np.ndarray) -> np.ndarray:
    """w [32, 288, 8] -> SBUF-layout lhsT blocks [128, 36*128].

    W_full[k*8+t, p*8+n] = w[p, k, (n-t) % 8]; block (ki, m) at columns
    (ki*MCH+m)*128 holds W_full[ki*128+p, m*128+f].
    """
    num_r, num_c, bs = w.shape
    wfull = np.zeros((num_c * bs, num_r * bs), np.float32)
    for t in range(bs):
        for n in range(bs):
            wfull[t::bs, n::bs] = w[:, :, (n - t) % bs].T
    return np.ascontiguousarray(
        wfull.reshape(KCH, 128, MCH, 128)
        .transpose(1, 0, 2, 3)
        .reshape(128, KCH * MCH * 128)
    )


def _kernel_body(tc, x, wt, bias, out):
    nc = tc.nc
    with (
        tc.tile_pool(name="const", bufs=1) as const_pool,
        tc.tile_pool(name="xp", bufs=1) as xp_pool,
        tc.tile_pool(name="ob", bufs=4) as out_pool,
        tc.tile_pool(name="ps", bufs=4, space="PSUM") as psum_pool,
        tc.tile_pool(name="dps", bufs=1, space="PSUM") as dummy_pool,
    ):
        # All 36 stationary blocks side by side: [128, 36*128]
        wt_sb = const_pool.tile([128, KCH * MCH * 128], F32R)
        nc.gpsimd.dma_start(out=wt_sb[:], in_=wt[:, :])

        bias_sb = const_pool.tile([128, MCH], F32)
        nc.gpsimd.dma_start(out=bias_sb[:], in_=bias[:, :])

        # Pre-padded input, one tile per 128-channel chunk (contiguous DMA)
        xq = []
        for i in range(MCH):
            t = xp_pool.tile([128, HP * HP], F32R, tag=f"xp{i}")
            nc.gpsimd.dma_start(
                out=t[:], in_=x[ts(i, 128), :, :].rearrange("p h w -> p (h w)")
            )
            xq.append(t)

        # Wait-absorber matmuls: walrus allows a single sync wait on a
        # Matmult (fp32 LDWEIGHTS slot). Chain one tiny matmul per input
        # DMA so each PE instruction picks up exactly one new semaphore;
        # all real matmuls then start with their inputs already observed.
        dummies = []
        dps = dummy_pool.tile([128, 64], F32)  # out dtype f32 (PSUM)
        for src_t in (xq[0], xq[1], wt_sb):
            d = nc.tensor.matmul(
                dps[0:1, 0:64],
                lhsT=src_t[:, 0:1],
                rhs=src_t[:, 0:64],
                start=True,
                stop=True,
            )
            if dummies:
                _add_dep_helper(
                    d.ins, dummies[-1].ins, sync=False,
                    reason="absorber chain",
                )
            dummies.append(d)

        # Same trick for the scalar engine: absorb the bias-DMA wait.
        junk = out_pool.tile([128, 1], F32, tag="junk")
        dact = nc.scalar.activation(
            junk[:], bias_sb[:, 0:1], mybir.ActivationFunctionType.Identity,
        )

        def conv_tile(nt, ob):
            """Compute output row-tile nt into ob [128, MCH, NT]."""
            for m in range(MCH):
                ps = psum_pool.tile([128, NT], F32, tag="ps")
                n_mm = 0
                for kh in range(KK):
                    for kw in range(KK):
                        for cb in range(MCH):
                            ki = (kh * KK + kw) * MCH + cb
                            xv = xq[cb][:].rearrange("p (h w) -> p h w", h=HP)
                            rhs = xv[
                                :,
                                nt * ROWS_PER_TILE + kh:
                                nt * ROWS_PER_TILE + kh + ROWS_PER_TILE,
                                kw: kw + W,
                            ]
                            mm = nc.tensor.matmul(
                                ps[:],
                                lhsT=wt_sb[:, ts(ki * MCH + m, 128)],
                                rhs=rhs,
                                start=(n_mm == 0),
                                stop=(n_mm == KCH - 1),
                            )
                            if n_mm == 0:
                                _add_dep_helper(
                                    mm.ins, dummies[-1].ins, sync=False,
                                    reason="after absorbers",
                                )
                            n_mm += 1
                act = nc.scalar.activation(
                    ob[:, m, :], ps[:],
                    mybir.ActivationFunctionType.Identity,
                    bias=bias_sb[:, m: m + 1],
                )
                _add_dep_helper(
                    act.ins, dact.ins, sync=False, reason="after bias absorber",
                )

        # out viewed as [p(128), m(2), pix]: c = m*128 + p
        out_v = out.rearrange("(m p) h w -> p m (h w)", m=MCH)
        # Group row-tiles into 4 output DMAs so (with the 4 input DMAs) at
        # most 8 DMA queue procs are ever used: no queue reuse, one wait per
        # descriptor, and a tail drain the CTRL struct can encode.
        for grp in ((0, 1), (2, 3), (4, 5), (6,)):
            gob = out_pool.tile(
                [128, MCH, len(grp) * NT], F32, tag=f"ob{len(grp)}"
            )
            for gi, nt in enumerate(grp):
                conv_tile(nt, gob[:, :, gi * NT: (gi + 1) * NT])
            nc.scalar.dma_start(
                out=out_v[:, :, grp[0] * NT: (grp[-1] + 1) * NT], in_=gob[:]
            )


def _build_nc():
    nc = bacc.Bacc("TRN2", target_bir_lowering=False, debug=False)
    x = nc.dram_tensor("x", [C, HP, HP], F32R, kind="ExternalInput").ap()
    wt = nc.dram_tensor("wt", [128, KCH * MCH * 128], F32R, kind="ExternalInput").ap()
    bias = nc.dram_tensor("bias", [128, MCH], F32, kind="ExternalInput").ap()
    out = nc.dram_tensor("out", [C, H, W], F32, kind="ExternalOutput").ap()
    with tile.TileContext(nc) as tc:
        _kernel_body(tc, x, wt, bias, out)
    nc.compile()
    return nc


def kernel(x: np.ndarray, w: np.ndarray, b: np.ndarray) -> np.ndarray:
    global LAST_RESULT
    xp = np.pad(np.asarray(x, np.float32), ((0, 0), (0, 0), (1, 1), (1, 1)))
    xp = np.ascontiguousarray(xp)
    wt = _expand_weight(np.asarray(w, np.float32))
    b = np.ascontiguousarray(
        np.asarray(b, np.float32).reshape(MCH, 128).T
    )

    nc = _build_nc()
    in_maps = [{"x": xp[i], "wt": wt, "bias": b} for i in range(N_CORES)]
    res = run_bass_kernel_spmd(
        nc,
        in_maps,
        core_ids=list(range(N_CORES)),
        trace=bool(int(os.environ.get("KERNEL_PROFILE", "0"))),
    )
    LAST_RESULT = res
    return np.stack([res.results[i]["out"] for i in range(N_CORES)], axis=0)


# revision 25
# speedup vs baseline: 3.2513x; 1.2611x over previous
"""BCM_Conv2d_fft kernel for Trainium2 (8 NeuronCores, batch-parallel).

The reference is a block-circulant 3x3 conv computed via per-block
rfft/irfft over the channel-block axis (block size 8). Per-frequency the
block products are independent, so in a real-DFT channel basis the
256->256 channel mixing matrix of each conv tap is block-diagonal with
frequency groups {f0:32, f4:32, f1:64, f2:64, f3:64}. Grouping
{f0,f4,f1} -> chunk0 and {f2,f3} -> chunk1 makes every tap's mixing
matrix chunk-diagonal: the conv needs 9 matmuls per output tile per
chunk instead of 18 - half the direct-conv PE work.

Device pipeline per core (one image):
  1. fwd:  xhat = A @ x       per pixel (A = real-DFT, freq-major rows)
  2. conv: ohat = sum_pos M_pos @ shift(xhat)   (chunk-diagonal M)
  3. inv:  out  = Ainv @ ohat + b

A, M_pos, Ainv are built on host from the tiny weight w [32,288,8] and
shipped as float32r stationary blocks. Matmuls run in float32r (full PE
rate at N>=256). Sharding: batch B=8 -> one image per core.
"""

import os

import numpy as np

import concourse.bacc as bacc
import concourse.mybir as mybir
import concourse.tile as tile
from concourse.bass import ts
from concourse.bass_utils import run_bass_kernel_spmd

N_CORES = 8
C = 256
H = W = 56
HP = H + 2
KK = 3
BS = 8
L = H * W
RPT = 8                  # output rows per tile
NT = RPT * W             # 448 pixels per tile
N_TILES = L // NT        # 7
MCH = C // 128           # 2 channel chunks

F32 = mybir.dt.float32
F32R = mybir.dt.float32r

# weight block column indices in the packed wts tensor [128, 26*128]
FWD_BLK = lambda i, c: i * MCH + c            # i = in chunk, c = out chunk
CONV_BLK = lambda pos, c: 4 + pos * MCH + c
INV_BLK = lambda k, m: 22 + k * MCH + m
N_BLKS = 26

LAST_RESULT = None


def _freq_matrices(w: np.ndarray):
    """Build A [256,256], Ms (9x [256,256] chunk-diag), Ainv from w."""
    F = np.zeros((8, 8))
    FI = np.fft.rfft(np.eye(8), axis=-1)
    F[0] = FI[:, 0].real
    F[1], F[2] = FI[:, 1].real, FI[:, 1].imag
    F[3], F[4] = FI[:, 2].real, FI[:, 2].imag
    F[5], F[6] = FI[:, 3].real, FI[:, 3].imag
    F[7] = FI[:, 4].real

    def fm(bk, comp):
        if comp == 0:
            return bk
        if comp == 7:
            return 32 + bk
        if comp in (1, 2):
            return 64 + 2 * bk + (comp - 1)
        if comp in (3, 4):
            return 128 + 2 * bk + (comp - 3)
        return 192 + 2 * bk + (comp - 5)

    A = np.zeros((256, 256))
    for bk in range(32):
        for comp in range(8):
            A[fm(bk, comp), bk * 8:(bk + 1) * 8] = F[comp]
    Ainv = np.linalg.inv(A)

    wf = np.fft.rfft(w.astype(np.float64), axis=-1)  # [32, 288, 5]
    Ms = []
    for pos in range(9):
        M = np.zeros((256, 256))
        for pb in range(32):
            for kb in range(32):
                kc = pos * 32 + kb
                M[fm(pb, 0), fm(kb, 0)] += wf[pb, kc, 0].real
                M[fm(pb, 7), fm(kb, 7)] += wf[pb, kc, 4].real
                for fi in range(3):
                    re_i, im_i = 1 + 2 * fi, 2 + 2 * fi
                    Wr, Wi = wf[pb, kc, fi + 1].real, wf[pb, kc, fi + 1].imag
                    M[fm(pb, re_i), fm(kb, re_i)] += Wr
                    M[fm(pb, re_i), fm(kb, im_i)] += -Wi
                    M[fm(pb, im_i), fm(kb, re_i)] += Wi
                    M[fm(pb, im_i), fm(kb, im_i)] += Wr
        Ms.append(M)
    return A, Ms, Ainv


def _pack_weights(w: np.ndarray) -> np.ndarray:
    """-> [128, 26*128] float32: lhsT blocks for fwd, conv, inv stages."""
    A, Ms, Ainv = _freq_matrices(w)
    wts = np.zeros((128, N_BLKS * 128), np.float32)

    def put(idx, mat):  # mat [K=128, M=128] already transposed for lhsT
        wts[:, idx * 128:(idx + 1) * 128] = mat.astype(np.float32)

    sl = lambda i: slice(i * 128, (i + 1) * 128)
    for i in range(MCH):
        for c in range(MCH):
            put(FWD_BLK(i, c), A[sl(c), sl(i)].T)
    for pos in range(9):
        for c in range(MCH):
            put(CONV_BLK(pos, c), Ms[pos][sl(c), sl(c)].T)
    for k in range(MCH):
        for m in range(MCH):
            put(INV_BLK(k, m), Ainv[sl(m), sl(k)].T)
    return wts


def _kernel_body(tc, x, wts, bias, out):
    nc = tc.nc
    with (
        tc.tile_pool(name="const", bufs=1) as const_pool,
        tc.tile_pool(name="xp", bufs=1) as xp_pool,
        tc.tile_pool(name="xh", bufs=1) as xh_pool,
        tc.tile_pool(name="oh", bufs=6) as oh_pool,
        tc.tile_pool(name="ob", bufs=4) as ob_pool,
        tc.tile_pool(name="psf", bufs=2, space="PSUM") as psf_pool,
        tc.tile_pool(name="psc", bufs=4, space="PSUM") as psc_pool,
        tc.tile_pool(name="psi", bufs=2, space="PSUM") as psi_pool,
    ):
        # Stationary blocks, split into 3 DMAs so the fwd stage can start
        # before the conv/inv blocks arrive.
        wt_sb = const_pool.tile([128, N_BLKS * 128], F32R)
        nc.sync.dma_start(out=wt_sb[:, 0:4 * 128], in_=wts[:, 0:4 * 128])
        nc.sync.dma_start(out=wt_sb[:, 4 * 128:22 * 128],
                          in_=wts[:, 4 * 128:22 * 128])
        nc.sync.dma_start(out=wt_sb[:, 22 * 128:], in_=wts[:, 22 * 128:])
        blk = lambda idx: wt_sb[:, ts(idx, 128)]

        bias_sb = const_pool.tile([128, MCH], F32)
        nc.sync.dma_start(out=bias_sb[:], in_=bias[:, :])

        # Pre-padded input, split into 4 row-range DMAs per chunk so the
        # fwd stage starts after the first piece lands.
        xq = []
        row_splits = [0, 16, 31, 46, HP]
        for i in range(MCH):
            t = xp_pool.tile([128, HP * HP], F32R, tag=f"xp{i}")
            for r0, r1 in zip(row_splits[:-1], row_splits[1:]):
                nc.sync.dma_start(
                    out=t[:, r0 * HP:r1 * HP],
                    in_=x[ts(i, 128), r0:r1, :].rearrange("p h w -> p (h w)"),
                )
            xq.append(t)

        # xhat: frequency-basis transform of the whole padded image (the
        # borders of x are zero, so xhat borders transform to zero too).
        xhat = []
        for c in range(MCH):
            xh_t = xh_pool.tile([128, HP * HP], F32R, tag=f"xh{c}")
            xhat.append(xh_t)
        # padded-row ranges per fwd tile: 7 tiles of 8 rows + 1 of 2 rows
        fwd_rows = [(it * RPT, min(HP, (it + 1) * RPT)) for it in range(8)]

        def fwd_tile(it):
            """Transform padded pixel rows [r0, r1) of the image."""
            r0, r1 = fwd_rows[it]
            npx = (r1 - r0) * HP
            for c in range(MCH):
                ps = psf_pool.tile([128, RPT * HP], F32, tag="psf")
                for i in range(MCH):
                    rhs = xq[i][:, r0 * HP: r1 * HP]
                    nc.tensor.matmul(
                        ps[:, :npx], lhsT=blk(FWD_BLK(i, c)), rhs=rhs,
                        start=(i == 0), stop=(i == MCH - 1),
                    )
                nc.vector.tensor_copy(
                    xhat[c][:, r0 * HP: r1 * HP], ps[:, :npx]
                )

        def conv_inv_tile(nt, ob):
            """Freq-domain conv + inverse transform for output tile nt."""
            ohat = []
            for c in range(MCH):
                ps = psc_pool.tile([128, NT], F32, tag="psc")
                n_mm = 0
                for kh in range(KK):
                    for kw in range(KK):
                        pos = kh * KK + kw
                        xhv = xhat[c][:].rearrange("p (h w) -> p h w", h=HP)
                        rhs = xhv[
                            :, nt * RPT + kh: nt * RPT + kh + RPT, kw: kw + W
                        ]
                        nc.tensor.matmul(
                            ps[:], lhsT=blk(CONV_BLK(pos, c)), rhs=rhs,
                            start=(n_mm == 0), stop=(n_mm == KK * KK - 1),
                        )
                        n_mm += 1
                oh = oh_pool.tile([128, NT], F32R, tag="oh")
                nc.vector.tensor_copy(oh[:], ps[:])
                ohat.append(oh)
            for m in range(MCH):
                ps = psi_pool.tile([128, NT], F32, tag="psi")
                for k in range(MCH):
                    nc.tensor.matmul(
                        ps[:], lhsT=blk(INV_BLK(k, m)), rhs=ohat[k][:],
                        start=(k == 0), stop=(k == MCH - 1),
                    )
                nc.scalar.activation(
                    ob[:, m, :], ps[:],
                    mybir.ActivationFunctionType.Identity,
                    bias=bias_sb[:, m: m + 1],
                )

        # out viewed as [p(128), m(2), pix]: c = m*128 + p
        out_v = out.rearrange("(m p) h w -> p m (h w)", m=MCH)

        def emit_out(nt, ob):
            nc.sync.dma_start(out=out_v[:, :, ts(nt, NT)], in_=ob[:])

        # Interleave: fwd runs 2 tiles ahead of conv (conv tile nt reads
        # padded xhat rows [nt*8, nt*8+9] = fwd tiles nt and nt+1).
        fwd_tile(0)
        fwd_tile(1)
        for nt in range(N_TILES):
            if nt + 2 < len(fwd_rows):
                fwd_tile(nt + 2)
            ob = ob_pool.tile([128, MCH, NT], F32, tag="ob")
            conv_inv_tile(nt, ob)
            emit_out(nt, ob)


def _build_nc():
    nc = bacc.Bacc("TRN2", target_bir_lowering=False, debug=False)
    x = nc.dram_tensor("x", [C, HP, HP], F32R, kind="ExternalInput").ap()
    wts = nc.dram_tensor("wts", [128, N_BLKS * 128], F32R,
                         kind="ExternalInput").ap()
    bias = nc.dram_tensor("bias", [128, MCH], F32, kind="ExternalInput").ap()
    out = nc.dram_tensor("out", [C, H, W], F32, kind="ExternalOutput").ap()
    with tile.TileContext(nc) as tc:
        _kernel_body(tc, x, wts, bias, out)
    nc.compile()
    return nc


def kernel(x: np.ndarray, w: np.ndarray, b: np.ndarray) -> np.ndarray:
    global LAST_RESULT
    xp = np.pad(np.asarray(x, np.float32), ((0, 0), (0, 0), (1, 1), (1, 1)))
    xp = np.ascontiguousarray(xp)
    wts = _pack_weights(np.asarray(w, np.float32))
    b = np.ascontiguousarray(np.asarray(b, np.float32).reshape(MCH, 128).T)

    nc = _build_nc()
    in_maps = [{"x": xp[i], "wts": wts, "bias": b} for i in range(N_CORES)]
    res = run_bass_kernel_spmd(
        nc,
        in_maps,
        core_ids=list(range(N_CORES)),
        trace=bool(int(os.environ.get("KERNEL_PROFILE", "0"))),
    )
    LAST_RESULT = res
    return np.stack([res.results[i]["out"] for i in range(N_CORES)], axis=0)
